# revision 68
# baseline (speedup 1.0000x reference)
"""Trainium2 Bass kernel: Longformer-style windowed attention with rotary,
head-averaged K/V (step_attn), fused QKV/out projections.

Sharding: 8 cores = (batch 2) x (sequence-quarter 4). Each core computes its
512 output rows for all 16 heads. No collectives.

Speed strategy vs the bf16 baseline:
- Q projection and the K(mean) projection run in fp8e4m3 with the DoubleRow
  matmul perf mode (two contraction rows per partition => K=256 per matmul at
  0.5 cycles/row: 4x fewer PE cycles). Weights are pre-scaled on host
  (wq x32, wk x64) and unscaled during PSUM evacuation.
- Attention score matmuls also run fp8 DoubleRow over the d=128 contraction
  (2x). K is stored d-pair-packed [64,2,keys] (deinterleaved, stationary
  side); Q is read d-pair-packed via a uint16-view DMA transpose which leaves
  the pair interleaved (allowed on the moving side).
- hid arrives pre-transposed from host (feature-major) in bf16 + fp8, so no
  startup DMA transposes.
- Softmax normalizer (ones-matmul), PV matmul, and the out-projection stay
  bf16. Output is written bf16 and upcast on host.
"""

import sys

for _p in ("/opt/trn_rl_repo", "/root/.axon_site/_ro/trn_rl_repo"):
    if _p not in sys.path:
        sys.path.append(_p)

import numpy as np
import ml_dtypes

import concourse.bass as bass
import concourse.tile as tile
from concourse import bacc
import concourse.mybir as mybir
from concourse.bass_utils import run_bass_kernel_spmd

F32 = mybir.dt.float32
BF16 = mybir.dt.bfloat16
FP8 = mybir.dt.float8e4
U16 = mybir.dt.uint16
MUL = mybir.AluOpType.mult
ADD = mybir.AluOpType.add
SUB = mybir.AluOpType.subtract
DR = mybir.MatmulPerfMode.DoubleRow
EXP = mybir.ActivationFunctionType.Exp
COPY = mybir.ActivationFunctionType.Copy

H = 16
D = 128
ROT = 32
HALF = 16  # ROT // 2
WIN = 256
G = 64
BASE = 10000.0
S = 2048
HD = H * D
B = 2
NCORES = 8
QROWS = 512          # rows per core
NKV = 6              # kv key-tiles per core
KVG_ROWS = NKV * 128 + G  # 832
SCALE = 1.0 / float(np.sqrt(np.float32(D)))
WQS = 32.0           # host pre-scale on wq (fp8 range)
WKS = 64.0           # host pre-scale on wk
K8S = 4.0            # extra scale on fp8 k values


# ---------------------------------------------------------------- device ----

def build_nc():
    nc = bacc.Bacc("TRN2", target_bir_lowering=False, debug=False,
                   num_devices=NCORES)

    aps = {}
    def inp(name, shape, dt):
        aps[name] = nc.dram_tensor(name, shape, dt, kind="ExternalInput").ap()

    inp("hidT", [128, 7 * 16 * 128], BF16)      # [p, (st, kt, r)]
    inp("hid8", [128, 16 * KVG_ROWS], FP8)      # [p, (kt, r)]
    inp("wq8", [128, 8 * 16 * 256], FP8)        # [p, (cch, kt, c)], x32
    inp("wk8", [128, 16 * D], FP8)              # [p, (kt, c)], x64
    inp("wv", [128, 16 * D], BF16)              # [p, (kt, c)]
    inp("wo", [HD, HD], BF16)
    inp("pk128", [128, 8 * HALF + 2 * NKV * HALF + NKV], F32)
    inp("pk64", [G, 2 * HALF], F32)
    inp("pkb", [1, 2 * HD + 2 * D], BF16)       # bq*32 | bk*64 | bv | bo
    inp("mask_win", [128, 4, 3, 128], BF16)
    inp("mask_glob", [G, QROWS], BF16)
    aps["out"] = nc.dram_tensor("out", [QROWS, HD], BF16,
                                kind="ExternalOutput").ap()

    with tile.TileContext(nc) as tc:
        _build_tile(nc, tc, aps)
    nc.compile()
    return nc


def _build_tile(nc, tc, aps):
    from contextlib import ExitStack
    import os
    _PH = int(os.environ.get("KERNEL_PHASES", "4"))
    ctx = ExitStack()

    persist = ctx.enter_context(tc.tile_pool(name="persist", bufs=1))
    dpool = ctx.enter_context(tc.tile_pool(name="dpool", bufs=2, space="DRAM"))
    ps = ctx.enter_context(tc.tile_pool(name="ps", bufs=8, space="PSUM"))
    # right-side pools: ctxR lives through attention (hidT/wv feed the
    # interleaved v projection); ctxR2 is released before the attention pools
    ctxR = ExitStack()
    hidp = ctxR.enter_context(tc.tile_pool(name="hidp", bufs=1, side="right"))
    ctxR2 = ExitStack()
    hidp2 = ctxR2.enter_context(tc.tile_pool(name="hidp2", bufs=1, side="right"))
    epool = ctxR2.enter_context(tc.tile_pool(name="evac", bufs=3, side="right"))

    # ---------------- persistent tiles
    hidT = hidp.tile([128, 7, 16, 128], BF16, tag="hidT")
    wv_sb = hidp.tile([128, 16, D], BF16, tag="wv_sb")
    hid8 = hidp2.tile([128, 16, KVG_ROWS], FP8, tag="hid8")
    wq8_sb = hidp2.tile([128, 8, 16, 256], FP8, tag="wq8_sb")
    wk8_sb = hidp2.tile([128, 16, D], FP8, tag="wk8_sb")
    q8 = hidp2.tile([128, 4, HD], FP8, tag="q8")
    qrot = hidp2.tile([128, 4, H, ROT], BF16, tag="qrot")
    kst = hidp2.tile([128, NKV, D], BF16, tag="kst")     # k after evac
    kg_sb = hidp2.tile([G, D], BF16, tag="kg_sb")
    k8_sb = hidp2.tile([128, NKV, 2 * D], FP8, tag="k8_sb")  # x4, d dup
    k8g_sb = hidp2.tile([G, 2 * D], FP8, tag="k8g_sb")
    kT8i = hidp2.tile([128, KVG_ROWS], U16, tag="kT8i")

    kv_sb = persist.tile([128, NKV, D], BF16, tag="kv_sb")   # v values
    kvg_sb = persist.tile([G, D], BF16, tag="kvg_sb")        # v glob
    kT8d = persist.tile([128, 2, KVG_ROWS], FP8, tag="kT8d")
    qT8u = persist.tile([128, 8, QROWS], U16, tag="qT8u")
    wo_sb = persist.tile([128, H, HD], BF16, tag="wo_sb")
    ones_c = persist.tile([128, 1], BF16, tag="ones_c")   # column (K=128, M=1)
    ones_r = persist.tile([1, 128], BF16, tag="ones_r")   # row (K=1, M=128)
    pk128 = persist.tile([128, 8 * HALF + 2 * NKV * HALF + NKV], F32,
                         tag="pk128")
    pk64 = persist.tile([G, 2 * HALF], F32, tag="pk64")
    pkb = persist.tile([1, 2 * HD + 2 * D], BF16, tag="pkb")
    mw_sb = persist.tile([128, 4, 3, 128], BF16, tag="mw")
    mg_sb = persist.tile([G, QROWS], BF16, tag="mg")
    cq_sb = pk128[:, 0:64].rearrange("p (so r) -> p so r", r=HALF)
    sq_sb = pk128[:, 64:128].rearrange("p (so r) -> p so r", r=HALF)
    ckv_sb = pk128[:, 128:224].rearrange("p (t r) -> p t r", r=HALF)
    skv_sb = pk128[:, 224:320].rearrange("p (t r) -> p t r", r=HALF)
    am_sb = pk128[:, 320:326]
    cg_sb = pk64[:, 0:HALF]
    sg_sb = pk64[:, HALF:2 * HALF]
    bq_sb = pkb[:, 0:HD]                       # x32
    bk_sb = pkb[:, HD:HD + D]                  # x64
    bv_sb = pkb[:, HD + D:HD + 2 * D]
    bo_sb = pkb[:, HD + 2 * D:2 * HD + 2 * D]

    # DRAM scratch
    q_d8 = [dpool.tile([QROWS // 2, HD], FP8, tag=f"q_d8{i}",
                       name=f"q_d8{i}") for i in range(2)]
    kk_d8 = dpool.tile([KVG_ROWS, 2 * D], FP8, tag="kk_d8")

    # ---------------- small loads (order = DMA priority: k/q path first)
    nc.gpsimd.memset(ones_c[:], 1.0)
    nc.gpsimd.memset(ones_r[:], 1.0)
    nc.sync.dma_start(out=wk8_sb[:], in_=aps["wk8"])
    # interleave hid8 kt-quad chunks (k-proj + q-proj lhs) with the wq8
    # column chunks so both streams finish early and PE never starves
    h8v = aps["hid8"].rearrange("p (t r) -> p t r", r=KVG_ROWS)
    w8v = aps["wq8"].rearrange("p (ch t c) -> p ch t c", t=16, c=256)
    nc.sync.dma_start(out=hid8[:, 0:4, :], in_=h8v[:, 0:4, :])
    for nm, t in (("pkb", pkb), ("pk128", pk128), ("pk64", pk64)):
        nc.sync.dma_start(out=t[:], in_=aps[nm])
    for q4 in range(1, 4):
        nc.sync.dma_start(out=hid8[:, 4 * q4:4 * q4 + 4, :],
                          in_=h8v[:, 4 * q4:4 * q4 + 4, :])
        nc.sync.dma_start(out=wq8_sb[:, q4 - 1, :, :], in_=w8v[:, q4 - 1, :, :])
    for ch in range(3, 8):
        nc.sync.dma_start(out=wq8_sb[:, ch, :, :], in_=w8v[:, ch, :, :])

    # ---------------- k projection (fp8 DoubleRow), one psum tile per st
    pk_t = []
    for st in range(NKV + 1):
        m = 128 if st < NKV else G
        t = ps.tile([128, 512], F32, tag="ps", name=f"pk{st}")
        pk_t.append((t, m))
    for j in range(8):
        for st in range(NKV + 1):
            t, m = pk_t[st]
            nc.tensor.matmul(t[:m, 0:D],
                             hid8[:, 2 * j:2 * j + 2, st * 128:st * 128 + m],
                             wk8_sb[:, 2 * j:2 * j + 2, :],
                             start=(j == 0), stop=False, perf_mode=DR)
    for st in range(NKV + 1):
        t, m = pk_t[st]
        nc.tensor.matmul(t[:m, 0:D], ones_r[:, :m], bk_sb[:],
                         start=False, stop=True)
    # evacs with 1/WKS scale (true k)
    for st in range(NKV):
        t, m = pk_t[st]
        nc.scalar.activation(kst[:, st, :], t[:, 0:D], COPY, scale=1.0 / WKS)
    tg, _ = pk_t[NKV]
    nc.scalar.activation(kg_sb[:], tg[:G, 0:D], COPY, scale=1.0 / WKS)

    # rotary on k (in-place, f32 temps): x1' = x1*c - x2*s ; x2' = x2*c + x1*s
    def rotary(x1, x2, c, s, shape, tag):
        t1 = epool.tile(shape, F32, tag=tag + "1", name=tag + "1")
        t2 = epool.tile(shape, F32, tag=tag + "2", name=tag + "2")
        nc.vector.tensor_tensor(out=t1[:], in0=x1, in1=s, op=MUL)
        nc.vector.tensor_tensor(out=t2[:], in0=x2, in1=s, op=MUL)
        nc.vector.tensor_tensor(out=x1, in0=x1, in1=c, op=MUL)
        nc.vector.tensor_tensor(out=x1, in0=x1, in1=t2[:], op=SUB)
        nc.vector.tensor_tensor(out=x2, in0=x2, in1=c, op=MUL)
        nc.vector.tensor_tensor(out=x2, in0=x2, in1=t1[:], op=ADD)

    rotary(kst[:, :, 0:HALF], kst[:, :, HALF:2 * HALF],
           ckv_sb[:], skv_sb[:], [128, NKV, HALF], "rkv")
    rotary(kg_sb[:, 0:HALF], kg_sb[:, HALF:2 * HALF],
           cg_sb[:], sg_sb[:], [G, HALF], "rg")

    # convert k -> fp8 (x K8S), duplicated d-block for the u16 transpose trick
    for half in range(2):
        nc.scalar.activation(k8_sb[:, :, half * D:(half + 1) * D], kst[:],
                             COPY, scale=K8S)
        nc.scalar.activation(k8g_sb[:, half * D:(half + 1) * D], kg_sb[:],
                             COPY, scale=K8S)
    # roundtrip: kk_d8 [832, 256] -> u16 transpose -> kT8i -> deinterleave
    nc.scalar.dma_start(out=kk_d8[0:NKV * 128, :]
                        .rearrange("(t p) c -> p t c", p=128),
                        in_=k8_sb[:])
    nc.scalar.dma_start(out=kk_d8[NKV * 128:KVG_ROWS, :], in_=k8g_sb[:])
    nc.scalar.dma_start(out=kT8i[:], in_=kk_d8[:].bitcast(U16), transpose=True)
    for hb in range(2):
        src = kT8i[64 * hb:64 * (hb + 1), :].bitcast(FP8) \
            .rearrange("p (r i) -> p i r", i=2)
        nc.vector.tensor_copy(kT8d[64 * hb:64 * (hb + 1), :, :], src)

    # ---------------- q projection (fp8 DoubleRow) -> q8 [128p, 4so, 2048c]
    if _PH < 2:
        o_dummy = persist.tile([128, HD], BF16, tag="o_dummy")
        nc.gpsimd.memset(o_dummy[:], 0.0)
        for Lp in range(4):
            nc.sync.dma_start(out=aps["out"][Lp * 128:(Lp + 1) * 128, :],
                              in_=o_dummy[:])
        ctxR2.close()
        ctxR.close()
        ctx.close()
        return
    # wv / hidT / masks stream: anchored on the LAST wq8 chunk's arrival so
    # these transfers fill the DMA idle window during qproj compute without
    # ever jumping ahead of the wq8 stream.
    hTv = aps["hidT"].rearrange("p (s t r) -> p s t r", t=16, r=128)
    nc.sync.dma_start(out=hidT[:, 0:3, :, :], in_=hTv[:, 0:3, :, :])
    nc.sync.dma_start(out=wv_sb[:],
                      in_=aps["wv"].rearrange("p (t c) -> p t c", c=D))
    nc.sync.dma_start(out=hidT[:, 6:7, :, :], in_=hTv[:, 6:7, :, :])
    nc.sync.dma_start(out=hidT[:, 3:6, :, :], in_=hTv[:, 3:6, :, :])
    nc.sync.dma_start(out=mw_sb[:], in_=aps["mask_win"])
    nc.sync.dma_start(out=mg_sb[:], in_=aps["mask_glob"])

    # ---------------- q projection in two so-halves: the so0/so1 rotary +
    # roundtrip + transposes overlap the so2/so3 matmul passes on PE
    q8v = q8[:].rearrange("p so (h c) -> p so h c", c=D)
    for sh in range(2):
        qd_u16 = q_d8[sh][:].bitcast(U16)      # [256, 1024]
        for cc in range(4):
            pq = [ps.tile([128, 512], F32, tag="ps", name=f"pq{sh}_{cc}_{sol}")
                  for sol in range(2)]
            for j in range(8):
                for sol in range(2):
                    so = 2 * sh + sol
                    lhs = hid8[:, 2 * j:2 * j + 2,
                               256 + so * 128:256 + so * 128 + 128]
                    for nh in range(2):
                        nc.tensor.matmul(
                            pq[sol][:, nh * 256:(nh + 1) * 256],
                            lhs,
                            wq8_sb[:, 2 * cc + nh, 2 * j:2 * j + 2, :],
                            start=(j == 0 and nh == 0), stop=False,
                            perf_mode=DR)
            for sol in range(2):
                so = 2 * sh + sol
                t = pq[sol]
                nc.tensor.matmul(t[:], ones_r[:],
                                 bq_sb[:, cc * 512:(cc + 1) * 512],
                                 start=False, stop=True)
                # full evac to fp8 (rot cols overwritten after rotary)
                nc.scalar.activation(q8[:, so, cc * 512:(cc + 1) * 512],
                                     t[:], COPY, scale=1.0 / WQS)
                # rot cols to bf16 staging (true scale) on DVE
                tv = t[:].rearrange("p (h c) -> p h c", c=D)
                nc.vector.tensor_scalar_mul(
                    qrot[:, so, 4 * cc:4 * cc + 4, :],
                    tv[:, :, 0:ROT], 1.0 / WQS)
        # rotary for this half (DVE + Pool in parallel), per-so writes
        for sol in range(2):
            so = 2 * sh + sol
            eng = nc.vector if sol == 0 else nc.gpsimd
            x1 = qrot[:, so, :, 0:HALF]
            x2 = qrot[:, so, :, HALF:ROT]
            c = cq_sb[:, so:so + 1, :].to_broadcast([128, H, HALF])
            s = sq_sb[:, so:so + 1, :].to_broadcast([128, H, HALF])
            t1 = epool.tile([128, H, HALF], F32, tag="rq1", name=f"rq1_{so}")
            t2 = epool.tile([128, H, HALF], F32, tag="rq2", name=f"rq2_{so}")
            eng.tensor_tensor(out=t1[:], in0=x1, in1=s, op=MUL)
            eng.tensor_tensor(out=t2[:], in0=x2, in1=s, op=MUL)
            eng.tensor_tensor(out=x1, in0=x1, in1=c, op=MUL)
            eng.tensor_tensor(out=x2, in0=x2, in1=c, op=MUL)
            eng.tensor_tensor(out=q8v[:, so, :, 0:HALF], in0=x1,
                              in1=t2[:], op=SUB)
            eng.tensor_tensor(out=q8v[:, so, :, HALF:ROT], in0=x2,
                              in1=t1[:], op=ADD)
            nc.sync.dma_start(
                out=q_d8[sh][sol * 128:(sol + 1) * 128, :]
                    .rearrange("(o p) c -> p o c", p=128),
                in_=q8[:, so:so + 1, :])
        for hp in range(8):
            nc.sync.dma_start(
                out=qT8u[:, hp, sh * 256:(sh + 1) * 256],
                in_=qd_u16[:, hp * 128:(hp + 1) * 128],
                transpose=True)

    # v projection for one st tile (issued interleaved with attention below)
    def v_proj_st(st):
        m = 128 if st < NKV else G
        t = ps.tile([128, 512], F32, tag="ps", name=f"pv{st}")
        for kt in range(16):
            nc.tensor.matmul(t[:m, 0:D], hidT[:, st, kt, 0:m],
                             wv_sb[:, kt, :], start=(kt == 0), stop=False)
        nc.tensor.matmul(t[:m, 0:D], ones_r[:, :m], bv_sb[:],
                         start=False, stop=True)
        if st < NKV:
            nc.scalar.activation(kv_sb[:, st, :], t[:, 0:D], COPY)
        else:
            nc.scalar.activation(kvg_sb[:], t[:G, 0:D], COPY)

    # wo stream (4 chunks of 4 heads). Dummy writes anchored on the so1
    # rotary output gate the DMAs so they cannot queue-jump ahead of the
    # wq8/hidT streams; each chunk DMA overwrites its dummy byte.
    wov = aps["wo"].rearrange("(h p) n -> p h n", p=128)
    anchor = q8[:, 1:2, 0:1].to_broadcast([128, 16, 1])
    nc.gpsimd.tensor_tensor(out=wo_sb[:, :, 0:1], in0=anchor, in1=anchor,
                            op=MUL)
    for hq in range(8):
        nc.sync.dma_start(out=wo_sb[:, 2 * hq:2 * hq + 2, :],
                          in_=wov[:, 2 * hq:2 * hq + 2, :])
    ctxR2.close()

    # ---------------- fused attention + out-projection, per block L
    if _PH < 3:
        o_dummy = persist.tile([128, HD], BF16, tag="o_dummy")
        nc.gpsimd.memset(o_dummy[:], 0.0)
        for Lp in range(4):
            nc.sync.dma_start(out=aps["out"][Lp * 128:(Lp + 1) * 128, :],
                              in_=o_dummy[:])
        ctxR.close()
        ctx.close()
        return
    wexp = ctx.enter_context(tc.tile_pool(name="wexp", bufs=8))
    rzp = ctx.enter_context(tc.tile_pool(name="rzp", bufs=4))
    opool = ctx.enter_context(tc.tile_pool(name="opool", bufs=2))
    aT_tiles = [None] * 4

    def qrhs(h, L):
        sl = qT8u[64 * (h % 2):64 * (h % 2) + 64, h // 2,
                  L * 128:(L + 1) * 128]
        return sl.bitcast(FP8).rearrange("p (r i) -> p i r", i=2)

    # heads grouped by parity so every score matmul in a PSUM group uses the
    # same base partition (tile_position churn inside a group is illegal)
    HSETS = [[0, 2, 4, 6], [1, 3, 5, 7], [8, 10, 12, 14], [9, 11, 13, 15]]

    def attn_scores(L, hg):
        heads = HSETS[hg]
        hb = heads[0] % 2
        pw = []
        for t in range(3):
            p_t = ps.tile([128, 512], F32, tag="ps", name=f"p_t{L}_{hg}_{t}")
            ptv = p_t[:].rearrange("p (h r) -> p h r", r=128)
            for hh, h in enumerate(heads):
                nc.tensor.matmul(
                    ptv[:, hh, :],
                    kT8d[64 * hb:64 * hb + 64, :,
                         (L + t) * 128:(L + t + 1) * 128],
                    qrhs(h, L),
                    start=(hh == 0), stop=(hh == 3), perf_mode=DR)
            pw.append(p_t)
        p_g = ps.tile([128, 512], F32, tag="ps", name=f"p_g{L}_{hg}")
        pgv = p_g[:G, :].rearrange("g (h r) -> g h r", r=128)
        for hh, h in enumerate(heads):
            nc.tensor.matmul(
                pgv[:, hh, :],
                kT8d[64 * hb:64 * hb + 64, :, NKV * 128:KVG_ROWS],
                qrhs(h, L),
                start=(hh == 0), stop=(hh == 3), perf_mode=DR)

        # exp + masks run on Act/Pool/DVE while PE moves on
        w_t = []
        for t in range(3):
            w = wexp.tile([128, 512], BF16, tag="wexp", name=f"w{L}_{hg}_{t}")
            nc.scalar.activation(w[:], pw[t][:], EXP,
                                 bias=am_sb[:, L + t:L + t + 1],
                                 scale=SCALE / K8S)
            eng = nc.gpsimd if t != 1 else nc.vector
            eng.tensor_tensor(
                out=w[:].rearrange("p (h s) -> p h s", s=128),
                in0=w[:].rearrange("p (h s) -> p h s", s=128),
                in1=mw_sb[:, L, t:t + 1, :].to_broadcast([128, 4, 128]),
                op=MUL)
            w_t.append(w)
        w_g = wexp.tile([G, 512], BF16, tag="wexpg", bufs=4,
                        name=f"wg{L}_{hg}")
        nc.scalar.activation(w_g[:], p_g[:G, :], EXP, scale=SCALE / K8S)
        nc.vector.tensor_tensor(
            out=w_g[:].rearrange("g (h s) -> g h s", s=128),
            in0=w_g[:].rearrange("g (h s) -> g h s", s=128),
            in1=mg_sb[:, L * 128:(L + 1) * 128]
                .rearrange("g (o s) -> g o s", o=1)
                .to_broadcast([G, 4, 128]),
            op=MUL)
        return w_t, w_g

    def attn_softmax(L, hg, w_t, w_g):
        aT = aT_tiles[L]
        pz = ps.tile([128, 512], F32, tag="ps", name=f"pz{L}_{hg}")
        for t in range(3):
            nc.tensor.matmul(pz[:1, :], ones_c[:], w_t[t][:],
                             start=(t == 0), stop=False)
        nc.tensor.matmul(pz[:1, :], ones_c[:G, :], w_g[:],
                         start=False, stop=True)
        z_sb = rzp.tile([1, 512], BF16, tag="z_sb", name=f"z{L}_{hg}")
        nc.vector.tensor_copy(z_sb[:], pz[:1, :])
        nc.tensor.matmul(pz[:], ones_r[:], z_sb[:], start=True, stop=True)
        rzb = rzp.tile([128, 512], F32, tag="rzb", name=f"rzb{L}_{hg}")
        nc.vector.reciprocal(out=rzb[:], in_=pz[:])

        po = ps.tile([128, 512], F32, tag="ps", name=f"po{L}_{hg}")
        for t in range(3):
            nc.tensor.matmul(po[:], kv_sb[:, L + t, :], w_t[t][:],
                             start=(t == 0), stop=False)
        nc.tensor.matmul(po[:], kvg_sb[:], w_g[:], start=False, stop=True)
        aTv = aT[:].rearrange("p (x par) r -> p x par r", par=2)
        nc.vector.tensor_tensor(
            out=aTv[:, 4 * (hg // 2):4 * (hg // 2) + 4, hg % 2, :],
            in0=po[:].rearrange("p (h s) -> p h s", s=128),
            in1=rzb[:].rearrange("p (h s) -> p h s", s=128),
            op=MUL)

    # pipelined: attention(L) interleaved with out-proj(L-1)
    oproj_state = {}

    def oproj_seg(Lp, seg):
        aT = aT_tiles[Lp]
        half = seg // 2
        if seg % 2 == 0:
            po2 = [ps.tile([128, 512], F32, tag="ps",
                           name=f"po2_{Lp}_{half}_{i}") for i in range(2)]
            oproj_state[(Lp, half)] = po2
        else:
            po2 = oproj_state[(Lp, half)]
        hs = range(8 * (seg % 2), 8 * (seg % 2) + 8)
        for h in hs:
            for j in range(2):
                ncn = 2 * half + j
                nc.tensor.matmul(po2[j][:], aT[:, h, :],
                                 wo_sb[:, h, ncn * 512:(ncn + 1) * 512],
                                 start=(h == 0), stop=False)
        if seg % 2 == 1:
            o_sb = oproj_state.setdefault(
                ("o", Lp), opool.tile([128, HD], BF16, tag="o_sb",
                                      name=f"o_sb{Lp}"))
            for j in range(2):
                ncn = 2 * half + j
                nc.tensor.matmul(po2[j][:], ones_r[:],
                                 bo_sb[:, ncn * 512:(ncn + 1) * 512],
                                 start=False, stop=True)
                nc.scalar.activation(o_sb[:, ncn * 512:(ncn + 1) * 512],
                                     po2[j][:], COPY)
                nc.sync.dma_start(
                    out=aps["out"][Lp * 128:(Lp + 1) * 128,
                                   ncn * 512:(ncn + 1) * 512],
                    in_=o_sb[:, ncn * 512:(ncn + 1) * 512])

    V_BEFORE = {0: [0, 1, 2, NKV], 1: [3], 2: [4], 3: [5]}
    iters = [(L, hg) for L in range(4) for hg in range(4)]
    pending = None
    for i, (L, hg) in enumerate(iters):
        if hg == 0:
            for st in V_BEFORE[L]:
                v_proj_st(st)
            aT_tiles[L] = wexp.tile([128, H, 128], BF16, tag="aT", bufs=3,
                                    name=f"aT{L}")
        w_t, w_g = attn_scores(L, hg)
        if pending is not None:
            attn_softmax(*pending)
        pending = (L, hg, w_t, w_g)
        if _PH >= 4 and L >= 1:
            oproj_seg(L - 1, hg)
    attn_softmax(*pending)
    if _PH >= 4:
        o_sb3 = opool.tile([128, HD], BF16, tag="o_sb", name="o_sb3f")
        for ncn in range(4):
            po3 = ps.tile([128, 512], F32, tag="ps", name=f"po3_{ncn}")
            for h in range(16):
                nc.tensor.matmul(po3[:], aT_tiles[3][:, h, :],
                                 wo_sb[:, h, ncn * 512:(ncn + 1) * 512],
                                 start=(h == 0), stop=False)
            nc.tensor.matmul(po3[:], ones_r[:],
                             bo_sb[:, ncn * 512:(ncn + 1) * 512],
                             start=False, stop=True)
            nc.scalar.activation(o_sb3[:, ncn * 512:(ncn + 1) * 512],
                                 po3[:], COPY)
            nc.sync.dma_start(
                out=aps["out"][3 * 128:4 * 128,
                               ncn * 512:(ncn + 1) * 512],
                in_=o_sb3[:, ncn * 512:(ncn + 1) * 512])
    else:
        o_dummy = persist.tile([128, HD], BF16, tag="o_dummy")
        nc.gpsimd.memset(o_dummy[:], 0.0)
        for Lp in range(4):
            nc.sync.dma_start(out=aps["out"][Lp * 128:(Lp + 1) * 128, :],
                              in_=o_dummy[:])

    ctxR.close()
    ctx.close()


# ------------------------------------------------------------------ host ----

_NC_CACHE = None


def _get_nc():
    global _NC_CACHE
    if _NC_CACHE is None:
        _NC_CACHE = build_nc()
    return _NC_CACHE


def make_in_maps(hidden_states, attention_mask, glob_idx, W_qkv, b_qkv, W_o, b_o):
    bf = ml_dtypes.bfloat16
    f8 = ml_dtypes.float8_e4m3
    hidden_states = np.asarray(hidden_states, np.float32)
    attention_mask = np.asarray(attention_mask, np.float32)
    glob_idx = np.asarray(glob_idx)
    W_qkv = np.asarray(W_qkv, np.float32)
    b_qkv = np.asarray(b_qkv, np.float32)
    W_o = np.asarray(W_o, np.float32)
    b_o = np.asarray(b_o, np.float32)

    w3 = W_qkv.reshape(HD, H, 3 * D)
    wq = np.ascontiguousarray(w3[:, :, :D].reshape(HD, HD))
    wk = w3[:, :, D:2 * D].mean(axis=1)
    wv = w3[:, :, 2 * D:].mean(axis=1)
    # wq8 [p, (col-chunk, kt, c)] so column chunks are contiguous
    wq8 = np.transpose((wq * WQS).reshape(16, 128, 8, 256), (1, 2, 0, 3)) \
        .reshape(128, 8 * 16 * 256).astype(f8)
    wk8 = np.transpose((wk * WKS).reshape(16, 128, D), (1, 0, 2)) \
        .reshape(128, 16 * D).astype(f8)
    wv_l = np.transpose(wv.reshape(16, 128, D), (1, 0, 2)) \
        .reshape(128, 16 * D).astype(bf)
    b3 = b_qkv.reshape(H, 3 * D)
    bq = np.ascontiguousarray(b3[:, :D].reshape(1, HD)) * WQS
    bk = b3[:, D:2 * D].mean(axis=0)[None, :] * WKS
    bv = b3[:, 2 * D:].mean(axis=0)[None, :]
    bo = b_o[None, :]
    pkb = np.concatenate([bq, bk, bv, bo], axis=1).astype(bf)
    wo = W_o.astype(bf)

    inv_freq = 1.0 / (BASE ** (np.arange(0, ROT, 2, dtype=np.float32) / ROT))
    freqs = np.arange(S, dtype=np.float32)[:, None] * inv_freq[None, :]  # [S,16]
    cos_all = np.cos(freqs).astype(np.float32)
    sin_all = np.sin(freqs).astype(np.float32)

    in_maps = []
    for c in range(NCORES):
        b, q = divmod(c, 4)
        t0 = 4 * q - 2
        tiles = [max(0, t0 + i) for i in range(NKV)]       # clipped content
        intended = [t0 + i for i in range(NKV)]
        kv_rows = np.concatenate([np.arange(t * 128, t * 128 + 128)
                                  for t in tiles])
        g_rows = glob_idx[b].astype(np.int64)
        rows = np.concatenate([kv_rows, g_rows])
        hid_c = np.ascontiguousarray(hidden_states[b][rows])   # [832, 2048]
        hidT_c = np.transpose(hid_c.reshape(KVG_ROWS, 16, 128), (2, 1, 0)) \
            .reshape(128, 16 * KVG_ROWS)
        # bf16 copy in [p, (st, kt, r)] layout, glob tile zero-padded to 128
        hid_pad = np.zeros((896, HD), np.float32)
        hid_pad[:KVG_ROWS] = hid_c
        hidT_st = np.transpose(hid_pad.reshape(7, 128, 16, 128), (3, 0, 2, 1)) \
            .reshape(128, 7 * 16 * 128)

        q_rows = np.arange(QROWS * q, QROWS * (q + 1))
        cos_q = cos_all[q_rows].reshape(4, 128, HALF).transpose(1, 0, 2).copy()
        sin_q = sin_all[q_rows].reshape(4, 128, HALF).transpose(1, 0, 2).copy()
        cos_kv = cos_all[kv_rows].reshape(NKV, 128, HALF).transpose(1, 0, 2).copy()
        sin_kv = sin_all[kv_rows].reshape(NKV, 128, HALF).transpose(1, 0, 2).copy()
        cos_g = cos_all[g_rows].copy()
        sin_g = sin_all[g_rows].copy()

        am = attention_mask[b, 0, 0]                        # [S]
        am_loc = am[kv_rows].reshape(NKV, 128).T.copy()     # [128, NKV]

        # window masks [128 key-p, 4 L, 3 t, 128 s]: valid iff
        # row-(WIN-1) <= key_pos <= row and the slot's intended tile exists
        mask_win = np.zeros((128, 4, 3, 128), np.float32)
        for L in range(4):
            rows_glb = QROWS * q + L * 128 + np.arange(128)          # [s]
            for t in range(3):
                it = intended[L + t]
                if it < 0:
                    continue
                key_pos = it * 128 + np.arange(128)                  # [p]
                valid = (key_pos[:, None] <= rows_glb[None, :]) & \
                        (key_pos[:, None] >= rows_glb[None, :] - (WIN - 1))
                mask_win[:, L, t, :] = valid
        # glob mask [64, 512]: row >= WIN and glob_idx < row - WIN
        rows_glb = QROWS * q + np.arange(QROWS)
        mask_glob = ((rows_glb[None, :] >= WIN) &
                     (g_rows[:, None] < rows_glb[None, :] - WIN)).astype(np.float32)

        pk128 = np.concatenate(
            [cos_q.reshape(128, 64), sin_q.reshape(128, 64),
             cos_kv.reshape(128, 96), sin_kv.reshape(128, 96),
             am_loc], axis=1).astype(np.float32)
        pk64 = np.concatenate([cos_g, sin_g], axis=1).astype(np.float32)
        in_maps.append({
            "hidT": hidT_st.astype(bf),
            "hid8": hidT_c.astype(f8),
            "wq8": wq8, "wk8": wk8, "wv": wv_l, "wo": wo,
            "pk128": pk128, "pk64": pk64, "pkb": pkb,
            "mask_win": mask_win.astype(bf),
            "mask_glob": mask_glob.astype(bf),
        })
    return in_maps


def kernel(hidden_states, attention_mask, glob_idx, W_qkv, b_qkv, W_o, b_o):
    nc = _get_nc()
    in_maps = make_in_maps(hidden_states, attention_mask, glob_idx,
                           W_qkv, b_qkv, W_o, b_o)
    res = run_bass_kernel_spmd(nc, in_maps, core_ids=list(range(NCORES)))
    out = np.empty((B, S, HD), np.float32)
    for c in range(NCORES):
        b, q = divmod(c, 4)
        out[b, QROWS * q:QROWS * (q + 1), :] = \
            res.results[c]["out"].astype(np.float32)
    return out
